# revision 1
# baseline (speedup 1.0000x reference)
"""Trainium2 Bass kernel for nn_Cov_EBFLayer.

Math: out[b,o] = exp(-quad[o,b]),
  quad[o,b] = diff^T P_o diff,  diff = c_o - x_b,  P_o = B_o B_o^T  (PSD Gram)
            = x^T P x - 2 v_o^T x + q3_o,   v = P c,  q3 = c^T P c
            = sum_{d,f} P[o,d,f] * (x_d x_f)  - 2 sum_d v[o,d] x_d + q3_o

Kernel strategy (per core, batch-sharded 8 x 1024):
  - Degree-2 feature map: G^T[(d,f), b] = x_d * x_f built on DVE from a
    PE-broadcast operand (indicator matmuls) times a stacked xT operand.
  - P computed on device: 256 Gram matmuls betasT_o^T @ betasT_o -> PSUM,
    ACT copies to SBUF in [d, (f,o)] layout, DRAM round trip re-reads it as
    weight chunks W_c[(d,f), o] (contiguous per partition).
  - Main contraction: 33 accumulating matmuls per (o-half, b-tile) PSUM tile:
    32 quadratic chunks (K=128) + 1 augmented chunk (K=65: linear + const).
  - Epilogue: one ACT Exp (scale=-1) straight out of PSUM, DMA out as [O, Bsh].
Host does layout-only prep (transposes) + the tiny linear-term prep
(w = B^T c, v = B w, q3 = w.w : ~2M MACs = 0.01% of model FLOPs).
"""

import sys
from contextlib import ExitStack

import numpy as np

sys.path.insert(0, "/opt/trn_rl_repo")

import concourse.bass as bass  # noqa: E402
import concourse.tile as tile  # noqa: E402
from concourse import bacc, mybir  # noqa: E402
from concourse import bass_utils  # noqa: E402
from concourse._compat import with_exitstack  # noqa: E402

B, D, O, NCORES = 8192, 64, 256, 8
BSH = B // NCORES  # 1024 per-core batch shard
NQC = D // 2  # 32 quadratic chunks, each (2 d's) x (64 f's) = 128 partitions
BT = 512  # b-tile (one PSUM bank of fp32)
NBT = BSH // BT  # 2
F32 = mybir.dt.float32
F16 = mybir.dt.float16


@with_exitstack
def _kernel(ctx: ExitStack, tc, outT, xT, betasT, indc, lin):
    nc = tc.nc

    cpool = ctx.enter_context(tc.tile_pool(name="const", bufs=1))
    gpool = ctx.enter_context(tc.tile_pool(name="gtiles", bufs=6))
    opool = ctx.enter_context(tc.tile_pool(name="outs", bufs=4))
    dpool = ctx.enter_context(tc.tile_pool(name="dram", bufs=1, space="DRAM"))
    ppool = ctx.enter_context(tc.tile_pool(name="psum_p", bufs=2, space="PSUM"))
    apool = ctx.enter_context(tc.tile_pool(name="psum_a", bufs=2, space="PSUM"))
    qpool = ctx.enter_context(tc.tile_pool(name="psum_q", bufs=4, space="PSUM"))

    # ---- resident inputs (xb first: unblocks the PE warm-up) ----
    xb = cpool.tile([128, BSH], F16)  # [xT; xT] stacked
    nc.sync.dma_start(xb[0:D, :], xT[:])
    nc.sync.dma_start(xb[D : 2 * D, :], xT[:])
    sb_betasT = cpool.tile([D, O * D], F16)  # [e, (o,d)]
    nc.sync.dma_start(sb_betasT[:], betasT[:])
    sb_indc = cpool.tile([D, NQC * 128], F16)
    nc.sync.dma_start(sb_indc[:], indc[:])
    g_aug = cpool.tile([D + 1, BSH], F16)  # [xT; ones]
    nc.sync.dma_start(g_aug[0:D, :], xT[:])
    nc.gpsimd.memset(g_aug[D : D + 1, :], 1.0)
    w_aug = cpool.tile([D + 1, O], F16)  # [-2 v^T; q3]
    nc.sync.dma_start(w_aug[:], lin[:])

    # ---- PE warm-up: ~3.5us of back-to-back matmuls so HAM reaches K=8/8
    # while input DMAs are still in flight. Results are overwritten by the
    # real accumulation (start=True resets PSUM). ----
    pq = {}
    for oh in range(2):
        for bt in range(NBT):
            pq[(oh, bt)] = qpool.tile(
                [128, BT], F32, name=f"pq_{oh}_{bt}", tag="pq"
            )
    for i in range(16):
        nc.tensor.matmul(
            pq[(i % 2, (i // 2) % 2)][:],
            xb[0:D, 0:128],
            xb[0:D, 0:BT],
            start=True,
            stop=True,
        )

    # ---- phase P: P_o = B_o^T B_o  (Gram), to SBUF layout [d, (f, o)] ----
    # processed in two o-halves so the DRAM round trip pipelines
    p_sb = cpool.tile([D, D * O], F16)  # [d, (f, o)]
    p_sb_v = p_sb[:].rearrange("d (f o) -> d o f", o=O)  # iter (o, f)
    p_dram = dpool.tile([D, D * O], F16)
    p_dram_v = p_dram[:].rearrange("d (f o) -> d f o", o=O)
    # two weight tiles, one per o-half, so main matmuls of half h depend
    # only on half h's P round trip
    w_half = [
        cpool.tile([128, NQC * 128], F16, name=f"w_half{h}", tag=f"w_half{h}")
        for h in range(2)
    ]

    # device o-index (oo) permutation: even real o -> oo=o/2, odd -> oo=128+o/2.
    # Host un-permutes output rows / permutes lin columns to match.
    p_sb_fo = p_sb[:].rearrange("d (f o) -> d f o", o=O)
    for half in range(2):
        for blk in range(16):  # 4 o-pairs (8 real o's) per PSUM bank
            pp = ppool.tile([128, 4 * 128], F32)
            for t in range(4):
                tt = half * 64 + blk * 4 + t  # pair index: covers o = 2tt, 2tt+1
                bsl = sb_betasT[:, tt * 2 * D : (tt * 2 + 2) * D]  # [64, 128]
                nc.tensor.matmul(
                    pp[:, t * 128 : (t + 1) * 128], bsl, bsl, start=True, stop=True
                )
            # diag blocks -> p_sb[d, f*O + oo]; within half h:
            # even o's at oo=128h+blk*4+t, odd at oo=128h+64+blk*4+t
            t0 = half * 128 + blk * 4
            pv_lo = pp[0:D, :].rearrange("d (t b) -> d b t", b=128)
            pv_hi = pp[D:128, :].rearrange("d (t b) -> d b t", b=128)
            eng = nc.scalar if blk % 2 == 0 else nc.vector
            if blk % 2 == 0:
                eng.activation(
                    p_sb_fo[:, :, t0 : t0 + 4],
                    pv_lo[:, 0:D, :],
                    mybir.ActivationFunctionType.Copy,
                )
                eng.activation(
                    p_sb_fo[:, :, 64 + t0 : 64 + t0 + 4],
                    pv_hi[:, D:128, :],
                    mybir.ActivationFunctionType.Copy,
                )
            else:
                eng.tensor_copy(p_sb_fo[:, :, t0 : t0 + 4], pv_lo[:, 0:D, :])
                eng.tensor_copy(
                    p_sb_fo[:, :, 64 + t0 : 64 + t0 + 4], pv_hi[:, D:128, :]
                )
        # round trip through DRAM for this half: oo in [128h, 128h+128)
        oo0 = half * 128
        nc.sync.dma_start(
            p_dram_v[:, :, oo0 : oo0 + 128], p_sb_fo[:, :, oo0 : oo0 + 128]
        )
        # coalesced W reads: one DMA per j, all 32 chunks of this half
        p_dram_j = p_dram[:].rearrange("(c j) (f o) -> j f c o", j=2, o=O)
        w_v = w_half[half][:].rearrange("p (c o) -> p c o", o=128)
        for j in range(2):
            nc.sync.dma_start(
                w_v[j * D : (j + 1) * D, :, :],
                p_dram_j[j, :, :, oo0 : oo0 + 128],
            )

    # ---- main: G chunks + accumulating matmuls ----
    for c in range(NQC + 1):
        for bt in range(NBT):
            if c < NQC:
                pa = apool.tile([128, BT], F32)
                nc.tensor.matmul(
                    pa[:],
                    sb_indc[:, c * 128 : (c + 1) * 128],
                    xb[0:D, bt * BT : (bt + 1) * BT],
                    start=True,
                    stop=True,
                )
                g = gpool.tile([128, BT], F16, tag="g")
                nc.vector.tensor_mul(g[:], pa[:], xb[:, bt * BT : (bt + 1) * BT])
                rhs = g[:]
            else:
                rhs = g_aug[:, bt * BT : (bt + 1) * BT]
            for oh in range(2):
                if c < NQC:
                    lhsT = w_half[oh][:, c * 128 : (c + 1) * 128]
                else:
                    lhsT = w_aug[:, oh * 128 : (oh + 1) * 128]
                nc.tensor.matmul(
                    pq[(oh, bt)][:],
                    lhsT,
                    rhs,
                    start=(c == 0),
                    stop=(c == NQC),
                )

    # ---- epilogue: out = exp(-quad) ----
    for oh in range(2):
        for bt in range(NBT):
            ob = opool.tile([128, BT], F32)
            nc.scalar.activation(
                ob[:],
                pq[(oh, bt)][:],
                mybir.ActivationFunctionType.Exp,
                scale=-1.0,
            )
            nc.sync.dma_start(
                outT[oh * 128 : (oh + 1) * 128, bt * BT : (bt + 1) * BT], ob[:]
            )


_CACHE = {}


def _build():
    if "nc" in _CACHE:
        return _CACHE["nc"], _CACHE["aps"]
    nc = bacc.Bacc(
        "TRN2", target_bir_lowering=False, debug=False, num_devices=NCORES
    )
    xT = nc.dram_tensor("xT", [D, BSH], F16, kind="ExternalInput").ap()
    betasT = nc.dram_tensor("betasT", [D, O * D], F16, kind="ExternalInput").ap()
    indc = nc.dram_tensor("indc", [D, NQC * 128], F16, kind="ExternalInput").ap()
    lin = nc.dram_tensor("lin", [D + 1, O], F16, kind="ExternalInput").ap()
    outT = nc.dram_tensor("outT", [O, BSH], F32, kind="ExternalOutput").ap()
    with tile.TileContext(nc) as tc:
        _kernel(tc, outT, xT, betasT, indc, lin)
    nc.compile()
    _CACHE["nc"] = nc
    _CACHE["aps"] = (xT, betasT, indc, lin, outT)
    return nc, _CACHE["aps"]


def _host_prep(x, centers, betas):
    x = np.asarray(x, np.float32)
    betas = np.asarray(betas, np.float32)
    c = np.asarray(centers, np.float32).reshape(O, D)
    # layout-only transposes
    betasT = np.ascontiguousarray(betas.transpose(2, 0, 1).reshape(D, O * D)).astype(np.float16)
    # indicator constant for PE row-broadcast: indc[d, c*128+p] = [d == 2c + p//64]
    dgrid = 2 * (np.arange(NQC)[:, None] * 1) + (np.arange(128)[None, :] // D)
    indc = (np.arange(D)[:, None, None] == dgrid[None, :, :]).astype(np.float32)
    indc = np.ascontiguousarray(indc.reshape(D, NQC * 128)).astype(np.float16)
    # tiny linear-term prep: w = B^T c, v = B w, q3 = w.w  (~2M MACs)
    w = np.einsum("ofe,of->oe", betas, c)
    v = np.einsum("ode,oe->od", betas, w)
    q3 = np.einsum("oe,oe->o", w, w)
    lin = np.concatenate([-2.0 * v.T, q3[None, :]], axis=0).astype(np.float16)
    # device o-permutation: even o -> o//2, odd o -> 128 + o//2
    operm = np.array(
        [128 * (o // 128) + (o % 2) * 64 + (o % 128) // 2 for o in range(O)]
    )
    lin_d = np.empty_like(lin)
    lin_d[:, operm] = lin
    lin = np.ascontiguousarray(lin_d)
    xT_shards = [
        np.ascontiguousarray(x[i * BSH : (i + 1) * BSH].T).astype(np.float16) for i in range(NCORES)
    ]
    return xT_shards, betasT, indc, lin


def _run(x, centers, betas, trace=False):
    nc, (xT, betasT_ap, indc_ap, lin_ap, outT) = _build()
    xT_shards, betasT, indc, lin = _host_prep(x, centers, betas)
    in_maps = [
        {
            xT.name: xT_shards[i],
            betasT_ap.name: betasT,
            indc_ap.name: indc,
            lin_ap.name: lin,
        }
        for i in range(NCORES)
    ]
    res = bass_utils.run_bass_kernel_spmd(
        nc, in_maps, core_ids=list(range(NCORES)), trace=trace
    )
    operm = np.array(
        [128 * (o // 128) + (o % 2) * 64 + (o % 128) // 2 for o in range(O)]
    )
    out = np.concatenate(
        [np.asarray(res.results[i][outT.name])[operm, :].T for i in range(NCORES)],
        axis=0,
    )
    return out.astype(np.float32), res


def kernel(x, centers, betas):
    out, _ = _run(x, centers, betas, trace=False)
    return out



# revision 16
# speedup vs baseline: 2.9220x; 2.9220x over previous
"""Trainium2 Bass kernel for nn_Cov_EBFLayer.

Math: out[b,o] = exp(-quad[o,b]),
  quad[o,b] = diff^T P_o diff,  diff = c_o - x_b,  P_o = B_o B_o^T  (PSD Gram)
            = sum_{d,f} P[o,d,f] x_d x_f - 2 v_o^T x + q3_o,  v = P c, q3 = c^T P c

Kernel strategy (per core, batch-sharded 8 x 1024):
  Symmetric-pair feature map over cyclic offsets: unordered pairs {d, f} at
  cyclic distance k are covered once by offset-k rows (d, (d+k)%64), k=1..32.
  17 feature chunks of 128 rows x 1024 batch:
    - 16 "pair" chunks, offsets (2j+1, 2j+2) in the two 64-row halves.
      Built per-chunk via one of three paths (engine balancing):
        V: DVE tensor_mul on partition-offset views of xb2=[x;x]
           (a cyclic rotation of x by k is just xb2[k:k+64])
        S/P: SBUF->SBUF DMA materializes the rotated operand ("slot"),
           then one full-width tensor_mul on DVE (S) or GPSIMD (P)
        A: PE indicator matmul computes s = x_d + x_f into PSUM, ACT engine
           squares it: u = (x_d+x_f)^2; host adjusts W (A=coeff/2) and folds
           the unwanted x^2 cross terms into the diagonal weights.
    - 1 "misc" chunk: rows 0:64 = x_d^2 (DVE), rows 64:128 = x_d (direct DMA).
  Main contraction: per chunk, 2 accumulating matmuls (o-halves) into
  PSUM quad tiles [128, 1024]. Epilogue: ACT Exp(scale=-1, bias=-q3) -> fp16.
Host does weight prep only (P = beta beta^T, W chunk layout, v, q3): O(model),
independent of batch.
"""

import sys
from contextlib import ExitStack

import numpy as np

sys.path.insert(0, "/opt/trn_rl_repo")

import concourse.bass as bass  # noqa: E402
import concourse.tile as tile  # noqa: E402
from concourse import bacc, mybir  # noqa: E402
from concourse import bass_utils  # noqa: E402
from concourse._compat import with_exitstack  # noqa: E402

B, D, O, NCORES = 8192, 64, 256, 8
BSH = B // NCORES  # 1024 per-core batch shard
BT = 512  # matmul free-dim tile (one PSUM bank of fp32)
F32 = mybir.dt.float32
F16 = mybir.dt.float16

# Accumulation-order chunk sequence. Pair chunk at list position j (skipping
# the misc entry) gets cyclic offsets (2j+1, 2j+2); paths: M=misc,
# S=slot+DVE mul, P=slot+GPSIMD mul, A=PE sum + ACT square,
# D=PE sum + DVE square (from PSUM).
_PATTERN = ["M", "S", "S", "A", "S", "A", "S", "S", "A", "P", "S", "A", "S", "P", "A", "S", "A"]
SEQ = []
_pj = 0
for _p in _PATTERN:
    if _p == "M":
        SEQ.append(("M", 0, 0))
    else:
        SEQ.append((_p, 2 * _pj + 1, 2 * _pj + 2))
        _pj += 1
NCH = len(SEQ)  # 17
NACT = sum(1 for s in SEQ if s[0] in ("A", "D"))


@with_exitstack
def _kernel(ctx: ExitStack, tc, outT, xT, wts, ind, bias):
    nc = tc.nc

    cpool = ctx.enter_context(tc.tile_pool(name="const", bufs=1))
    gpool = ctx.enter_context(tc.tile_pool(name="gtiles", bufs=5))
    opool = ctx.enter_context(tc.tile_pool(name="outs", bufs=2))
    qpool = ctx.enter_context(tc.tile_pool(name="psum_q", bufs=4, space="PSUM"))
    spool = ctx.enter_context(tc.tile_pool(name="psum_s", bufs=4, space="PSUM"))

    # ---- resident inputs (xb2 first: unblocks warm-up + all builds) ----
    xb2 = cpool.tile([128, BSH], F16)  # [x; x] stacked
    nc.sync.dma_start(xb2[0:D, :], xT[:])
    nc.sync.dma_start(xb2[D : 2 * D, :], xT[:])
    g_misc = cpool.tile([128, BSH], F16)  # [x^2; x]
    nc.sync.dma_start(g_misc[D:128, :], xT[:])
    w_sb = cpool.tile([128, NCH * O], F16)
    nc.sync.dma_start(w_sb[:], wts[:])
    i_sb = cpool.tile([D, NACT * 128], F16)
    nc.sync.dma_start(i_sb[:], ind[:])
    b_sb = cpool.tile([128, 2], F32)  # -q3 per o-half
    nc.sync.dma_start(b_sb[:], bias[:])

    # PSUM bank = 2 KB/partition, so matmul outputs are [128, 512] fp32 max;
    # quad accumulates in 4 tiles (2 o-halves x 2 b-tiles).
    pq = {
        (oh, bt): qpool.tile([128, BT], F32, name=f"pq{oh}{bt}", tag="pq")
        for oh in range(2)
        for bt in range(2)
    }

    # ---- PE warm-up: back-to-back matmuls so HAM reaches 8/8 while input
    # DMAs are still in flight. Overwritten by the real accumulation. ----
    for i in range(12):
        nc.tensor.matmul(
            pq[(i % 2, (i // 2) % 2)][:],
            xb2[0:D, 0:128],
            xb2[0:D, 0:BT],
            start=True,
            stop=True,
        )

    # ---- slot builds: SBUF->SBUF DMA of rotated x views ----
    slots = {}
    for j, (p, k1, k2) in enumerate(SEQ):
        if p in ("S", "P"):
            t = cpool.tile([128, BSH], F16, name=f"slot{j}")
            nc.sync.dma_start(t[0:D, :], xb2[k1 : k1 + D, :])
            nc.sync.dma_start(t[D:128, :], xb2[k2 : k2 + D, :])
            slots[j] = t

    # misc diag half: x_d^2
    nc.vector.tensor_mul(g_misc[0:D, :], xb2[0:D, :], xb2[0:D, :])

    act_pos = [j for j, s in enumerate(SEQ) if s[0] in ("A", "D")]
    s_tiles = {}
    state = {"ind_ptr": 0, "squares_done": 0}

    def top_up_inds():
        # keep <=2 chunks of indicator matmuls in flight ahead of the squares
        while (
            state["ind_ptr"] < len(act_pos)
            and state["ind_ptr"] - state["squares_done"] < 2
        ):
            ai = state["ind_ptr"]
            j = act_pos[ai]
            ss = []
            for bt in range(2):
                s = spool.tile([128, BT], F32, tag="s")
                nc.tensor.matmul(
                    s[:],
                    i_sb[:, ai * 128 : (ai + 1) * 128],
                    xb2[0:D, bt * BT : (bt + 1) * BT],
                    start=True,
                    stop=True,
                )
                ss.append(s)
            s_tiles[j] = ss
            state["ind_ptr"] += 1

    # ---- main loop: build G chunk, 2 accumulating matmuls ----
    for j, (p, k1, k2) in enumerate(SEQ):
        top_up_inds()
        if p == "M":
            g = g_misc
        elif p in ("S", "P"):
            g = gpool.tile([128, BSH], F16, tag="g")
            eng = nc.vector if p == "S" else nc.gpsimd
            eng.tensor_mul(g[:], xb2[:], slots[j][:])
        else:  # A: square the pair-sums on ACT straight out of PSUM
            g = gpool.tile([128, BSH], F16, tag="g")
            for bt in range(2):
                nc.scalar.activation(
                    g[:, bt * BT : (bt + 1) * BT],
                    s_tiles[j][bt][:],
                    mybir.ActivationFunctionType.Square,
                )
            state["squares_done"] += 1
        for bt in range(2):
            for oh in range(2):
                nc.tensor.matmul(
                    pq[(oh, bt)][:],
                    w_sb[:, j * O + oh * 128 : j * O + oh * 128 + 128],
                    g[:, bt * BT : (bt + 1) * BT],
                    start=(j == 0),
                    stop=(j == NCH - 1),
                )

    # ---- epilogue: out = exp(-(quad + q3)) ----
    for oh in range(2):
        ob = opool.tile([128, BSH], F16, name=f"ob{oh}", tag="ob")
        for bt in range(2):
            nc.scalar.activation(
                ob[:, bt * BT : (bt + 1) * BT],
                pq[(oh, bt)][:],
                mybir.ActivationFunctionType.Exp,
                bias=b_sb[:, oh : oh + 1],
                scale=-1.0,
            )
        nc.sync.dma_start(outT[oh * 128 : (oh + 1) * 128, :], ob[:])


_CACHE = {}


def _build():
    if "nc" in _CACHE:
        return _CACHE["nc"], _CACHE["aps"]
    nc = bacc.Bacc(
        "TRN2", target_bir_lowering=False, debug=False, num_devices=NCORES
    )
    xT = nc.dram_tensor("xT", [D, BSH], F16, kind="ExternalInput").ap()
    wts = nc.dram_tensor("wts", [128, NCH * O], F16, kind="ExternalInput").ap()
    ind = nc.dram_tensor("ind", [D, NACT * 128], F16, kind="ExternalInput").ap()
    bias = nc.dram_tensor("bias", [128, 2], F32, kind="ExternalInput").ap()
    outT = nc.dram_tensor("outT", [O, BSH], F16, kind="ExternalOutput").ap()
    with tile.TileContext(nc) as tc:
        _kernel(tc, outT, xT, wts, ind, bias)
    nc.compile()
    _CACHE["nc"] = nc
    _CACHE["aps"] = (xT, wts, ind, bias, outT)
    return nc, _CACHE["aps"]


def _host_prep(x, centers, betas):
    x32 = np.asarray(x, np.float32)
    betas32 = np.asarray(betas, np.float32)
    cen = np.asarray(centers, np.float32).reshape(O, D)
    # weight prep: O(model), batch-independent
    P = np.matmul(betas32, betas32.transpose(0, 2, 1))  # [O, D, D]
    w = np.einsum("ofe,of->oe", betas32, cen)
    v = np.einsum("ode,oe->od", betas32, w)
    q3 = np.einsum("oe,oe->o", w, w)

    dd = np.arange(D)
    R = np.zeros((O, D), np.float32)  # x^2 corrections from A-chunks
    Wstack = np.zeros((NCH, 128, O), np.float32)
    Istack = []
    for j, (p, k1, k2) in enumerate(SEQ):
        if p == "M":
            continue
        for half, k in ((0, k1), (1, k2)):
            f = (dd + k) % D
            coeff = (2.0 if k < D // 2 else 1.0) * P[:, dd, f]  # [O, 64]
            if p in ("A", "D"):
                A_ = coeff * 0.5
                Wstack[j, half * D : (half + 1) * D, :] = A_.T
                R[:, dd] += A_
                R[:, f] += A_  # f is a permutation: indices unique
            else:
                Wstack[j, half * D : (half + 1) * D, :] = coeff.T
        if p in ("A", "D"):
            I = np.zeros((D, 128), np.float32)
            pp = np.arange(128)
            dcol = pp % D
            kcol = np.where(pp < D, k1, k2)
            I[dcol, pp] += 1.0
            I[(dcol + kcol) % D, pp] += 1.0
            Istack.append(I)
    mj = next(j for j, s in enumerate(SEQ) if s[0] == "M")
    Wstack[mj, 0:D, :] = (P[:, dd, dd] - R).T
    Wstack[mj, D:128, :] = (-2.0 * v).T

    wts = np.ascontiguousarray(
        Wstack.transpose(1, 0, 2).reshape(128, NCH * O)
    ).astype(np.float16)
    ind = np.ascontiguousarray(np.concatenate(Istack, axis=1)).astype(np.float16)
    bias = np.ascontiguousarray((-q3).reshape(2, 128).T).astype(np.float32)
    xT_shards = [
        np.ascontiguousarray(x32[i * BSH : (i + 1) * BSH].T).astype(np.float16)
        for i in range(NCORES)
    ]
    return xT_shards, wts, ind, bias


def _run(x, centers, betas, trace=False):
    nc, (xT, wts_ap, ind_ap, bias_ap, outT) = _build()
    xT_shards, wts, ind, bias = _host_prep(x, centers, betas)
    in_maps = [
        {
            xT.name: xT_shards[i],
            wts_ap.name: wts,
            ind_ap.name: ind,
            bias_ap.name: bias,
        }
        for i in range(NCORES)
    ]
    res = bass_utils.run_bass_kernel_spmd(
        nc, in_maps, core_ids=list(range(NCORES)), trace=trace
    )
    out = np.concatenate(
        [np.asarray(res.results[i][outT.name]).T for i in range(NCORES)],
        axis=0,
    )
    return out.astype(np.float32), res


def kernel(x, centers, betas):
    out, _ = _run(x, centers, betas, trace=False)
    return out
